# revision 27
# baseline (speedup 1.0000x reference)
"""CRF loss (nn_CRFLayer) on 8 Trainium2 NeuronCores — 3-segment, length-sorted kernel.

Strategy (pure data parallel over batch):
  B=4096 -> 8 cores x 512 seqs; per core 512 seqs = 4 groups x 128 columns.
  State TRANSPOSED: vT[(g,t), b'] in exp domain with global per-step shift K;
  per step ONE bf16 matmul (static block-diag exp(transitions)^T) + one DVE
  multiply with host-precomputed eF = exp(feats - K) (fp8 e5m2). Tag 31
  (STOP) never propagates, so row (g,31) stashes the group-sum captured at
  s=len(b) via the pad pattern e_31; fwd(b) = ln(total) + K*len(b).

  Two structural optimizations on top of the recurrence:
  1) TIME SPLIT: three segments / four concurrent lanes (true fwd F1, true
     adjoint B3, interior fwd F2 / adjoint B2 over the shared middle). The
     middle operator is rank-1 to machine precision, so
       total = (y.M u)*(w.M v)/(ones.u)  per (g,b')  -- three dot products.
  2) LENGTH SORT + WIDTH TRIM: sequences are dealt longest-first to
     (core, column), so at slot s only the first w[s] = ceil(#{len>=s}/32)
     columns are active. Every matmul and multiply shrinks to w[s] columns;
     frozen columns keep their stash in persistent in-place state tiles.
     Segment boundaries are chosen at runtime to balance lane wall-times.

  Gold score: host marshals pure index lookups (transition pairs + emission
  gather, minus K per valid step); device sums and subtracts. The batch mean
  is permutation-invariant, so no unsort is needed.
"""
import sys
import numpy as np

sys.path.insert(0, "/opt/trn_rl_repo")

B, S, T = 4096, 512, 32
START, STOP = 30, 31
NCORES = 8
BC = B // NCORES          # 512 sequences per core
G = 4                     # groups per core
P = 128                   # partitions
NSLOT = S + 1             # eF slots 0..512

_compiled = None
_plan = None


def _lead_chunks(total):
    """Geometric lead-in pieces then 43s: [6,12,25,43,...,rem]."""
    out = [6, 12, 25]
    left = total - 43
    while left > 43:
        out.append(43)
        left -= 43
    if left > 0:
        out.append(left)
    return out


def _make_plan(lengths):
    lengths = np.asarray(lengths).astype(np.int64)
    N = np.array([(lengths >= s).sum() for s in range(S + 2)])
    w = np.minimum(P, np.ceil(N / (NCORES * G)).astype(np.int64))
    w = np.maximum(w, 1)
    # empirical: lane round latency ~450+2.3w ns, DVE op ~180+0.8w ns
    lat = lambda ws: 450 + 2.3 * ws
    dve = lambda ws: 180 + 0.8 * ws
    best = None
    for M1 in range(60, 400, 4):
        for M2 in range(M1 + 40, 510, 4):
            f1 = sum(lat(w[s]) for s in range(1, M1 + 1))
            mid = sum(lat(w[s]) for s in range(M1, M2))
            b3 = sum(lat(w[s]) for s in range(M2, S + 1))
            dtot = (sum(dve(w[s]) for s in range(1, M1 + 1))
                    + 2 * sum(dve(w[s]) for s in range(M1, M2))
                    + sum(dve(w[s]) for s in range(M2, S + 1)))
            wall = max(f1, mid, b3, dtot)
            if best is None or wall < best[0]:
                best = (wall, M1, M2)
    _, M1, M2 = best
    return {"w": [int(x) for x in w], "M1": int(M1), "M2": int(M2)}


def _build_bass(plan):
    import concourse.bass as bass
    import concourse.mybir as mybir
    from concourse.tile import TileContext

    f32 = mybir.dt.float32
    bf16 = mybir.dt.bfloat16
    f8e5 = mybir.dt.float8e5
    AF = mybir.ActivationFunctionType
    ALU = mybir.AluOpType
    AX = mybir.AxisListType

    w = plan["w"]
    M1, M2 = plan["M1"], plan["M2"]
    NF1 = M1                  # efw1 slots 0..M1-1 (slot 0 = seed, unused in DVE)
    NMID = M2 - M1            # mid slots M1..M2-1 (j = slot-M1)
    NB3 = NSLOT - M2          # ebt3: j=0 -> slot 512 (seed, unused), j=r -> 512-r
    RF1, RF2, RB2, RB3 = M1, NMID, NMID - 1, NB3 - 1
    RMAX = max(RF1, RF2, RB2, RB3)

    nc = bass.Bass()
    efw1_h = nc.dram_tensor("efw1", [P, NF1, P], f8e5, kind="ExternalInput")
    mid_h = nc.dram_tensor("mid", [P, NMID, P], f8e5, kind="ExternalInput")
    ebt3_h = nc.dram_tensor("ebt3", [P, NB3, P], f8e5, kind="ExternalInput")
    x2_h = nc.dram_tensor("x2", [P, P], bf16, kind="ExternalInput")
    v0_h = nc.dram_tensor("v0", [P, P], bf16, kind="ExternalInput")
    y0_h = nc.dram_tensor("y0", [P, P], bf16, kind="ExternalInput")
    w0_h = nc.dram_tensor("w0", [P, P], bf16, kind="ExternalInput")
    m2_h = nc.dram_tensor("m2ext", [P, P], bf16, kind="ExternalInput")
    m2b_h = nc.dram_tensor("m2bext", [P, P], bf16, kind="ExternalInput")
    gsel_h = nc.dram_tensor("gsel", [P, G], f32, kind="ExternalInput")
    gcomb_h = nc.dram_tensor("gcomb", [P, G * S], f32, kind="ExternalInput")
    loss_h = nc.dram_tensor("loss_part", [1, 1], f32, kind="ExternalOutput")

    with TileContext(nc) as tc:
        with (
            tc.tile_pool(name="singles", bufs=1) as singles,
            tc.tile_pool(name="f1pool", bufs=4) as f1pool,
            tc.tile_pool(name="b3pool", bufs=4) as b3pool,
            tc.tile_pool(name="small", bufs=2) as small,
            tc.tile_pool(name="dots", bufs=1) as dots,
            tc.tile_pool(name="ps_fw", bufs=1, space="PSUM") as ps_fw,
            tc.tile_pool(name="ps_bw", bufs=1, space="PSUM") as ps_bw,
            tc.tile_pool(name="ps_f", bufs=1, space="PSUM") as ps_f,
        ):
            # ---- persistent in-place states (seeded by DMA) ----
            v_st = singles.tile([P, P], bf16)
            nc.sync.dma_start(out=v_st[:], in_=v0_h[:])
            y_st = singles.tile([P, P], bf16)
            nc.sync.dma_start(out=y_st[:], in_=y0_h[:])
            u_st = singles.tile([P, P], bf16)
            nc.sync.dma_start(out=u_st[:], in_=x2_h[:])
            w_st = singles.tile([P, P], bf16)
            nc.sync.dma_start(out=w_st[:], in_=w0_h[:])

            m2_sb = singles.tile([P, P], bf16)
            nc.sync.dma_start(out=m2_sb[:], in_=m2_h[:])
            m2b_sb = singles.tile([P, P], bf16)
            nc.sync.dma_start(out=m2b_sb[:], in_=m2b_h[:])
            gsel_sb = singles.tile([P, G], f32)
            nc.sync.dma_start(out=gsel_sb[:], in_=gsel_h[:])
            ones_sb = singles.tile([P, 1], f32)
            nc.vector.memset(ones_sb[:], 1.0)

            mid_sb = singles.tile([P, NMID, P], f8e5)

            def mid_dma(j0, ln):
                nc.sync.dma_start(out=mid_sb[:, j0:j0 + ln, :],
                                  in_=mid_h[:, j0:j0 + ln, :])

            f1_tiles, off = [], 0
            for chn in _lead_chunks(NF1):
                t = f1pool.tile([P, chn, P], f8e5, tag="f1k")
                f1_tiles.append((t, off, chn))
                off += chn
            b3_tiles, off = [], 0
            for chn in _lead_chunks(NB3):
                t = b3pool.tile([P, chn, P], f8e5, tag="b3k")
                b3_tiles.append((t, off, chn))
                off += chn

            gcomb_sb = singles.tile([P, G * S], f32)
            gred = singles.tile([P, 1], f32)

            def slot_of(tiles, s):
                for t, o, c in tiles:
                    if o <= s < o + c:
                        return t, s - o
                raise IndexError(s)

            f1_dma = {o: (t, c) for t, o, c in f1_tiles}
            b3_dma = {o: (t, c) for t, o, c in b3_tiles}
            issued = set()

            def maybe_dma(s, dmas, h, key):
                if s in dmas and (key, s) not in issued:
                    issued.add((key, s))
                    t, c = dmas[s]
                    nc.sync.dma_start(out=t[:], in_=h[:, s:s + c, :])

            # mid lead-in pieces: B2 side (from the top) and F2 side
            midb_lead = [(NMID - 7, 7), (NMID - 19, 12), (NMID - 29, 10)]
            midf_lead = [(0, 7), (7, 12), (19, 10)]
            j0, j1 = 29, NMID - 29   # remainder [29, NMID-29)
            rest = []
            lo, hi = j0, j1
            while lo < hi:
                take = min(28, hi - lo)
                rest.append((hi - take, take))  # B2 side first
                hi -= take
                if lo < hi:
                    take = min(28, hi - lo)
                    rest.append((lo, take))
                    lo += take

            # pre-issue round-robin
            mid_dma(*midb_lead[0]); maybe_dma(0, f1_dma, efw1_h, 'f')
            maybe_dma(0, b3_dma, ebt3_h, 'b'); mid_dma(*midf_lead[0])
            mid_dma(*midb_lead[1]); maybe_dma(6, f1_dma, efw1_h, 'f')
            maybe_dma(6, b3_dma, ebt3_h, 'b'); mid_dma(*midf_lead[1])
            mid_dma(*midb_lead[2]); maybe_dma(18, f1_dma, efw1_h, 'f')
            maybe_dma(18, b3_dma, ebt3_h, 'b'); mid_dma(*midf_lead[2])
            for i, pc in enumerate(rest[:2]):
                mid_dma(*pc)
            maybe_dma(43, f1_dma, efw1_h, 'f'); maybe_dma(43, b3_dma, ebt3_h, 'b')
            for pc in rest[2:]:
                mid_dma(*pc)

            ps_m1 = psu_m2 = None
            for r in range(1, RMAX + 1):
                maybe_dma(r + 43, f1_dma, efw1_h, 'f')
                maybe_dma(r + 43, b3_dma, ebt3_h, 'b')
                if r == RMAX // 2:
                    nc.sync.dma_start(out=gcomb_sb[:], in_=gcomb_h[:])

                if r <= RF1:
                    wd = w[r] if r <= RF1 - 1 else P  # boundary MM full width
                    psf1 = ps_fw.tile([P, P], f32, tag="psf1")
                    nc.tensor.matmul(psf1[:, 0:wd], lhsT=m2_sb[:],
                                     rhs=v_st[:, 0:wd], start=True, stop=True)
                if r <= RB3:
                    s = S - r
                    wdy = w[s]
                    psb3 = ps_bw.tile([P, P], f32, tag="psb3")
                    nc.tensor.matmul(psb3[:, 0:wdy], lhsT=m2b_sb[:],
                                     rhs=y_st[:, 0:wdy], start=True, stop=True)
                if r <= RF2:
                    su = M1 + r - 1
                    wdu = w[su]
                    psf2 = ps_fw.tile([P, P], f32, tag="psf2")
                    nc.tensor.matmul(psf2[:, 0:wdu], lhsT=m2_sb[:],
                                     rhs=u_st[:, 0:wdu], start=True, stop=True)
                if r <= RB2:
                    sw = M2 - 1 - r
                    wdw = w[sw]
                    psb2 = ps_bw.tile([P, P], f32, tag="psb2")
                    nc.tensor.matmul(psb2[:, 0:wdw], lhsT=m2b_sb[:],
                                     rhs=w_st[:, 0:wdw], start=True, stop=True)

                if r <= RF1 - 1:
                    t, j = slot_of(f1_tiles, r)
                    nc.vector.tensor_tensor(out=v_st[:, 0:wd],
                                            in0=psf1[:, 0:wd],
                                            in1=t[:, j, 0:wd], op=ALU.mult)
                elif r == RF1:
                    ps_m1 = psf1
                if r <= RB3:
                    t, j = slot_of(b3_tiles, r)
                    nc.vector.tensor_tensor(out=y_st[:, 0:wdy],
                                            in0=psb3[:, 0:wdy],
                                            in1=t[:, j, 0:wdy], op=ALU.mult)
                if r <= RF2:
                    nc.vector.tensor_tensor(out=u_st[:, 0:wdu],
                                            in0=psf2[:, 0:wdu],
                                            in1=mid_sb[:, r - 1, 0:wdu],
                                            op=ALU.mult)
                    if r == RF2:
                        psu_m2 = ps_fw.tile([P, P], f32, tag="psf2")
                        nc.tensor.matmul(psu_m2[:], lhsT=m2_sb[:],
                                         rhs=u_st[:], start=True, stop=True)
                if r <= RB2:
                    nc.vector.tensor_tensor(out=w_st[:, 0:wdw],
                                            in0=psb2[:, 0:wdw],
                                            in1=mid_sb[:, NMID - 1 - r, 0:wdw],
                                            op=ALU.mult)

            # ---- epilogue ----
            nc.vector.tensor_reduce(gred[:], gcomb_sb[:], axis=AX.X, op=ALU.add)
            dA = dots.tile([P, P], f32)
            nc.vector.tensor_tensor(out=dA[:], in0=psu_m2[:], in1=y_st[:],
                                    op=ALU.mult)
            dC = dots.tile([P, P], f32)
            nc.vector.tensor_tensor(out=dC[:], in0=ps_m1[:], in1=w_st[:],
                                    op=ALU.mult)

            qA = ps_f.tile([G, P], f32, tag="psq")
            nc.tensor.matmul(qA[:], lhsT=gsel_sb[:], rhs=dA[:],
                             start=True, stop=True)
            lnA = small.tile([G, P], f32, tag="lnA")
            nc.scalar.activation(lnA[:], qA[:], AF.Ln)
            qC = ps_f.tile([G, P], f32, tag="psq")
            nc.tensor.matmul(qC[:], lhsT=gsel_sb[:], rhs=dC[:],
                             start=True, stop=True)
            lnC = small.tile([G, P], f32, tag="lnC")
            nc.scalar.activation(lnC[:], qC[:], AF.Ln)
            uf = dots.tile([P, P], f32)
            nc.scalar.copy(uf[:], u_st[:])
            qD = ps_f.tile([G, P], f32, tag="psq")
            nc.tensor.matmul(qD[:], lhsT=gsel_sb[:], rhs=uf[:],
                             start=True, stop=True)
            lnD = small.tile([G, P], f32, tag="lnD")
            nc.scalar.activation(lnD[:], qD[:], AF.Ln)

            fwd4 = small.tile([G, P], f32, tag="fwd4")
            nc.vector.tensor_add(fwd4[:], lnA[:], lnC[:])
            nc.vector.tensor_sub(fwd4[:], fwd4[:], lnD[:])
            fred = small.tile([G, 1], f32, tag="fred")
            nc.vector.tensor_reduce(fred[:], fwd4[:], axis=AX.X, op=ALU.add)

            psf1s = ps_f.tile([1, 1], f32, tag="pss")
            nc.tensor.matmul(psf1s[:], lhsT=fred[:], rhs=ones_sb[0:G, :],
                             start=True, stop=True)
            psg1 = ps_f.tile([1, 1], f32, tag="pss")
            nc.tensor.matmul(psg1[:], lhsT=gred[:], rhs=ones_sb[:],
                             start=True, stop=True)
            tf_sb = small.tile([1, 1], f32, tag="tf")
            nc.scalar.copy(tf_sb[:], psf1s[:])
            out_sb = small.tile([1, 1], f32, tag="outs")
            nc.vector.tensor_tensor(out=out_sb[:], in0=tf_sb[:], in1=psg1[:],
                                    op=ALU.subtract)
            nc.sync.dma_start(out=loss_h[:], in_=out_sb[:])

    return nc


def _estimate_k(feats, transitions):
    """Per-step log-growth of the forward recursion, from a 128-seq sample."""
    m = np.exp(transitions.T.astype(np.float64))  # m[frm, to]
    f = feats[:128].astype(np.float64)
    v = np.exp(transitions.T[START][None, :] + f[:, 0, :])
    v[:, 30:] = 0.0
    c = np.log(v.sum(1))
    v /= v.sum(1, keepdims=True)
    for s in range(1, S):
        v = (v @ m) * np.exp(f[:, s, :])
        v[:, 30:] = 0.0
        q = v.sum(1)
        c += np.log(q)
        v /= q[:, None]
    return float(c.mean() / S)


def _host_inputs(feats, tags, lengths, transitions, plan):
    import ml_dtypes
    bf16 = ml_dtypes.bfloat16
    f8 = ml_dtypes.float8_e5m2

    feats = np.asarray(feats, np.float32)
    tags = np.asarray(tags).astype(np.int64)
    lengths = np.asarray(lengths).astype(np.int64)
    transitions = np.asarray(transitions, np.float32)
    M1, M2 = plan["M1"], plan["M2"]

    K = _estimate_k(feats, transitions)

    # global longest-first deal: rank i -> core i%8, local r=i//8,
    # group r%4, column r//4  (columns die back-to-front, same on all cores)
    order = np.argsort(-lengths, kind="stable")
    perm = np.empty(B, np.int64)
    i = np.arange(B)
    core = i % NCORES
    lr = i // NCORES
    gg = lr % G
    col = lr // G
    perm[core * BC + gg * P + col] = order[i]
    feats = feats[perm]
    tags = tags[perm]
    lengths = lengths[perm]

    m = np.exp(transitions.T.astype(np.float64)).astype(np.float32)  # [frm,to]
    M2m = m.copy()
    M2m[:, STOP] = 1.0
    m2ext = np.zeros((P, P), np.float32)
    m2bext = np.zeros((P, P), np.float32)
    for g in range(G):
        m2ext[g * T:(g + 1) * T, g * T:(g + 1) * T] = M2m
        m2bext[g * T:(g + 1) * T, g * T:(g + 1) * T] = M2m.T
    m2ext = m2ext.astype(bf16)
    m2bext = m2bext.astype(bf16)

    gsel = np.zeros((P, G), np.float32)
    for g in range(G):
        gsel[g * T:(g + 1) * T, g] = 1.0

    rowt = np.arange(P) % T
    x2 = np.zeros((P, P), np.float32)
    x2[rowt <= 29, :] = 1.0
    x2 = x2.astype(bf16)

    flat = transitions.reshape(-1)
    tags_prev = np.concatenate(
        [np.full((B, 1), START, np.int64), tags[:, :-1]], axis=1)
    pairval = flat[(tags * T + tags_prev).reshape(-1)].reshape(B, S)
    emitval = np.take_along_axis(feats, tags[:, :, None], axis=2)[:, :, 0]
    smask = np.arange(S)[None, :] < lengths[:, None]
    gcomb = np.where(smask, pairval + emitval - K, 0.0).astype(np.float32)

    ef_all = np.exp(feats - np.float32(K))          # [B, S, T] f32
    init0 = np.exp(transitions.T[START][None, :] + feats[:, 0, :] - np.float32(K))
    init0[:, 30:] = 0.0

    per_core = []
    for c in range(NCORES):
        sl = slice(c * BC, (c + 1) * BC)
        len_c = lengths[sl]
        ef_c = ef_all[sl]
        eft = np.zeros((P, NSLOT, P), np.float32)
        src = ef_c.reshape(G, P, S, T).transpose(0, 3, 2, 1)
        eft_v = src.reshape(P, S, P)
        vmask = (np.arange(NSLOT)[None, :] < len_c[:, None])
        vm = vmask.reshape(G, P, NSLOT).transpose(0, 2, 1).reshape(
            G, 1, NSLOT, P) * np.ones((1, T, 1, 1))
        vm = vm.reshape(P, NSLOT, P)
        eft[:, 1:S, :] = np.where(vm[:, 1:S, :] > 0, eft_v[:, 1:S, :], 0.0)
        eft[rowt >= 30, :, :] = 0.0
        pad = (vm[:, :, :] == 0)
        r31 = (rowt == STOP)
        eft[np.ix_(r31, np.arange(NSLOT))] = np.where(
            pad[r31], 1.0, eft[r31])
        i0 = init0[sl].reshape(G, P, T).transpose(0, 2, 1).reshape(P, P)
        eft[:, 0, :] = i0
        eft8 = np.clip(eft, 0.0, 57344.0).astype(f8)
        per_core.append({
            "efw1": np.ascontiguousarray(eft8[:, 0:M1, :]),
            "mid": np.ascontiguousarray(eft8[:, M1:M2, :]),
            "ebt3": np.ascontiguousarray(eft8[:, S:M2 - 1:-1, :]),
            "x2": x2,
            "v0": np.ascontiguousarray(eft[:, 0, :].astype(bf16)),
            "y0": np.ascontiguousarray(eft[:, S, :].astype(bf16)),
            "w0": np.ascontiguousarray(eft[:, M2 - 1, :].astype(bf16)),
            "m2ext": m2ext,
            "m2bext": m2bext,
            "gsel": gsel,
            "gcomb": np.ascontiguousarray(
                gcomb[sl].reshape(G, P, S).transpose(1, 0, 2).reshape(P, G * S)),
        })
    return per_core


def kernel(feats, tags, lengths, transitions):
    global _compiled, _plan
    from concourse.bass_utils import run_bass_kernel_spmd
    import waitfix_embedded  # noqa: F401  (installs on import)

    if _plan is None:
        _plan = _make_plan(lengths)
    if _compiled is None:
        _compiled = _build_bass(_plan)
    nc = _compiled
    in_maps = _host_inputs(feats, tags, lengths, transitions, _plan)
    res = run_bass_kernel_spmd(nc, in_maps, core_ids=list(range(NCORES)))
    total = np.float64(0.0)
    for r in res.results:
        total += np.float64(r["loss_part"][0, 0])
    return np.float32(total / B)


# ---- embedded waitfix module (kernel.py must be self-contained) ----
import types as _types  # noqa: E402

_wf_src = '''
import json

MAX_WAITS = 1

def split_sync_waits(bir_bytes, max_waits=MAX_WAITS):
    bir = json.loads(bir_bytes)
    n_split = 0
    for fn in bir["functions"]:
        for blk in fn["blocks"]:
            out = []
            for inst in blk["instructions"]:
                si = inst.get("sync_info")
                waits = (si or {}).get("on_wait") or []
                if len(waits) > max_waits:
                    k = 0
                    while len(waits) > max_waits:
                        chunk, waits = waits[:max_waits], waits[max_waits:]
                        out.append({
                            "debug": inst.get("debug", 0),
                            "engine": inst["engine"],
                            "ins": [], "is_reset_sema": False,
                            "name": inst["name"] + "-wsplit%d" % k,
                            "opcode": "NoOp", "outs": [],
                            "sync_info": {"on_update": [], "on_wait": chunk},
                        })
                        k += 1
                    si["on_wait"] = waits
                    n_split += 1
                out.append(inst)
            blk["instructions"] = out
    return json.dumps(bir).encode()

def install():
    import concourse.bass2jax as bass2jax
    if getattr(bass2jax, "_waitfix_installed", False):
        return
    orig = bass2jax.compile_bir_kernel
    def patched(bir_json, tmpdir, neff_name="file.neff"):
        return orig(split_sync_waits(bir_json), tmpdir, neff_name)
    bass2jax.compile_bir_kernel = patched
    bass2jax._waitfix_installed = True

install()
'''
if "waitfix_embedded" not in sys.modules:
    _mod = _types.ModuleType("waitfix_embedded")
    exec(_wf_src, _mod.__dict__)
    sys.modules["waitfix_embedded"] = _mod


if __name__ == "__main__":
    import refcache
    inputs, exp = refcache.load()
    out = kernel(**inputs)
    rel = abs(float(out) - float(exp)) / max(abs(float(exp)), 1e-9)
    print("kernel:", out, "expected:", exp, "rel err:", rel)


# revision 28
# speedup vs baseline: 1.3113x; 1.3113x over previous
"""CRF loss (nn_CRFLayer) on 8 Trainium2 NeuronCores — 3-segment, length-sorted kernel.

Strategy (pure data parallel over batch):
  B=4096 -> 8 cores x 512 seqs; per core 512 seqs = 4 groups x 128 columns.
  State TRANSPOSED: vT[(g,t), b'] in exp domain with global per-step shift K;
  per step ONE bf16 matmul (static block-diag exp(transitions)^T) + one DVE
  multiply with host-precomputed eF = exp(feats - K) (fp8 e5m2). Tag 31
  (STOP) never propagates, so row (g,31) stashes the group-sum captured at
  s=len(b) via the pad pattern e_31; fwd(b) = ln(total) + K*len(b).

  Two structural optimizations on top of the recurrence:
  1) TIME SPLIT: three segments / four concurrent lanes (true fwd F1, true
     adjoint B3, interior fwd F2 / adjoint B2 over the shared middle). The
     middle operator is rank-1 to machine precision, so
       total = (y.M u)*(w.M v)/(ones.u)  per (g,b')  -- three dot products.
  2) LENGTH SORT + WIDTH TRIM: sequences are dealt longest-first to
     (core, column), so at slot s only the first w[s] = ceil(#{len>=s}/32)
     columns are active. Every matmul and multiply shrinks to w[s] columns;
     frozen columns keep their stash in persistent in-place state tiles.
     Segment boundaries are chosen at runtime to balance lane wall-times.

  Gold score: host marshals pure index lookups (transition pairs + emission
  gather, minus K per valid step); device sums and subtracts. The batch mean
  is permutation-invariant, so no unsort is needed.
"""
import sys
import numpy as np

sys.path.insert(0, "/opt/trn_rl_repo")

B, S, T = 4096, 512, 32
START, STOP = 30, 31
NCORES = 8
BC = B // NCORES          # 512 sequences per core
G = 4                     # groups per core
P = 128                   # partitions
NSLOT = S + 1             # eF slots 0..512

_compiled = None
_plan = None


def _lead_chunks(total):
    """Geometric lead-in pieces then 43s: [6,12,25,43,...,rem]."""
    out = [6, 12, 25]
    left = total - 43
    while left > 43:
        out.append(43)
        left -= 43
    if left > 0:
        out.append(left)
    return out


def _make_plan(lengths):
    lengths = np.asarray(lengths).astype(np.int64)
    N = np.array([(lengths >= s).sum() for s in range(S + 2)])
    w = np.minimum(P, np.ceil(N / (NCORES * G)).astype(np.int64))
    w = np.maximum(w, 1)
    # empirical: lane round latency ~650+0.8w ns, DVE sustained ~177 ns/op
    lat = lambda ws: 650 + 0.8 * ws
    dve = lambda ws: 177.0
    best = None
    for M1 in range(60, 400, 4):
        for M2 in range(M1 + 40, 510, 4):
            f1 = sum(lat(w[s]) for s in range(1, M1 + 1))
            mid = sum(lat(w[s]) for s in range(M1, M2))
            b3 = sum(lat(w[s]) for s in range(M2, S + 1))
            dtot = (sum(dve(w[s]) for s in range(1, M1 + 1))
                    + 2 * sum(dve(w[s]) for s in range(M1, M2))
                    + sum(dve(w[s]) for s in range(M2, S + 1)))
            wall = max(f1, mid, b3, dtot)
            if best is None or wall < best[0]:
                best = (wall, M1, M2)
    _, M1, M2 = best
    return {"w": [int(x) for x in w], "M1": int(M1), "M2": int(M2)}


def _build_bass(plan):
    import concourse.bass as bass
    import concourse.mybir as mybir
    from concourse.tile import TileContext

    f32 = mybir.dt.float32
    bf16 = mybir.dt.bfloat16
    f8e5 = mybir.dt.float8e5
    AF = mybir.ActivationFunctionType
    ALU = mybir.AluOpType
    AX = mybir.AxisListType

    w = plan["w"]
    M1, M2 = plan["M1"], plan["M2"]
    NF1 = M1                  # efw1 slots 0..M1-1 (slot 0 = seed, unused in DVE)
    NMID = M2 - M1            # mid slots M1..M2-1 (j = slot-M1)
    NB3 = NSLOT - M2          # ebt3: j=0 -> slot 512 (seed, unused), j=r -> 512-r
    RF1, RF2, RB2, RB3 = M1, NMID, NMID - 1, NB3 - 1
    RMAX = max(RF1, RF2, RB2, RB3)

    nc = bass.Bass()
    efw1_h = nc.dram_tensor("efw1", [P, NF1, P], f8e5, kind="ExternalInput")
    mid_h = nc.dram_tensor("mid", [P, NMID, P], f8e5, kind="ExternalInput")
    ebt3_h = nc.dram_tensor("ebt3", [P, NB3, P], f8e5, kind="ExternalInput")
    x2_h = nc.dram_tensor("x2", [P, P], bf16, kind="ExternalInput")
    v0_h = nc.dram_tensor("v0", [P, P], bf16, kind="ExternalInput")
    y0_h = nc.dram_tensor("y0", [P, P], bf16, kind="ExternalInput")
    w0_h = nc.dram_tensor("w0", [P, P], bf16, kind="ExternalInput")
    m2_h = nc.dram_tensor("m2ext", [P, P], bf16, kind="ExternalInput")
    m2b_h = nc.dram_tensor("m2bext", [P, P], bf16, kind="ExternalInput")
    gsel_h = nc.dram_tensor("gsel", [P, G], f32, kind="ExternalInput")
    gcomb_h = nc.dram_tensor("gcomb", [P, G * S], f32, kind="ExternalInput")
    loss_h = nc.dram_tensor("loss_part", [1, 1], f32, kind="ExternalOutput")

    with TileContext(nc) as tc:
        with (
            tc.tile_pool(name="singles", bufs=1) as singles,
            tc.tile_pool(name="f1pool", bufs=4) as f1pool,
            tc.tile_pool(name="b3pool", bufs=4) as b3pool,
            tc.tile_pool(name="small", bufs=2) as small,
            tc.tile_pool(name="dots", bufs=1) as dots,
            tc.tile_pool(name="ps_fw", bufs=1, space="PSUM") as ps_fw,
            tc.tile_pool(name="ps_bw", bufs=1, space="PSUM") as ps_bw,
            tc.tile_pool(name="ps_f", bufs=1, space="PSUM") as ps_f,
        ):
            # ---- persistent in-place states (seeded by DMA) ----
            v_st = singles.tile([P, P], bf16)
            nc.sync.dma_start(out=v_st[:], in_=v0_h[:])
            y_st = singles.tile([P, P], bf16)
            nc.sync.dma_start(out=y_st[:], in_=y0_h[:])
            u_st = singles.tile([P, P], bf16)
            nc.sync.dma_start(out=u_st[:], in_=x2_h[:])
            w_st = singles.tile([P, P], bf16)
            nc.sync.dma_start(out=w_st[:], in_=w0_h[:])

            m2_sb = singles.tile([P, P], bf16)
            nc.sync.dma_start(out=m2_sb[:], in_=m2_h[:])
            m2b_sb = singles.tile([P, P], bf16)
            nc.sync.dma_start(out=m2b_sb[:], in_=m2b_h[:])
            gsel_sb = singles.tile([P, G], f32)
            nc.sync.dma_start(out=gsel_sb[:], in_=gsel_h[:])
            ones_sb = singles.tile([P, 1], f32)
            nc.vector.memset(ones_sb[:], 1.0)

            mid_sb = singles.tile([P, NMID, P], f8e5)

            def mid_dma(j0, ln):
                nc.sync.dma_start(out=mid_sb[:, j0:j0 + ln, :],
                                  in_=mid_h[:, j0:j0 + ln, :])

            f1_tiles, off = [], 0
            for chn in _lead_chunks(NF1):
                t = f1pool.tile([P, chn, P], f8e5, tag="f1k")
                f1_tiles.append((t, off, chn))
                off += chn
            b3_tiles, off = [], 0
            for chn in _lead_chunks(NB3):
                t = b3pool.tile([P, chn, P], f8e5, tag="b3k")
                b3_tiles.append((t, off, chn))
                off += chn

            gcomb_sb = singles.tile([P, G * S], f32)
            gred = singles.tile([P, 1], f32)

            def slot_of(tiles, s):
                for t, o, c in tiles:
                    if o <= s < o + c:
                        return t, s - o
                raise IndexError(s)

            f1_dma = {o: (t, c) for t, o, c in f1_tiles}
            b3_dma = {o: (t, c) for t, o, c in b3_tiles}
            issued = set()

            def maybe_dma(s, dmas, h, key):
                if s in dmas and (key, s) not in issued:
                    issued.add((key, s))
                    t, c = dmas[s]
                    nc.sync.dma_start(out=t[:], in_=h[:, s:s + c, :])

            # mid lead-in pieces: B2 side (from the top) and F2 side
            midb_lead = [(NMID - 7, 7), (NMID - 19, 12), (NMID - 29, 10)]
            midf_lead = [(0, 7), (7, 12), (19, 10)]
            j0, j1 = 29, NMID - 29   # remainder [29, NMID-29)
            rest = []
            lo, hi = j0, j1
            while lo < hi:
                take = min(28, hi - lo)
                rest.append((hi - take, take))  # B2 side first
                hi -= take
                if lo < hi:
                    take = min(28, hi - lo)
                    rest.append((lo, take))
                    lo += take

            # pre-issue round-robin
            mid_dma(*midb_lead[0]); maybe_dma(0, f1_dma, efw1_h, 'f')
            maybe_dma(0, b3_dma, ebt3_h, 'b'); mid_dma(*midf_lead[0])
            mid_dma(*midb_lead[1]); maybe_dma(6, f1_dma, efw1_h, 'f')
            maybe_dma(6, b3_dma, ebt3_h, 'b'); mid_dma(*midf_lead[1])
            mid_dma(*midb_lead[2]); maybe_dma(18, f1_dma, efw1_h, 'f')
            maybe_dma(18, b3_dma, ebt3_h, 'b'); mid_dma(*midf_lead[2])
            for i, pc in enumerate(rest[:2]):
                mid_dma(*pc)
            maybe_dma(43, f1_dma, efw1_h, 'f'); maybe_dma(43, b3_dma, ebt3_h, 'b')
            for pc in rest[2:]:
                mid_dma(*pc)

            ps_m1 = psu_m2 = None
            for r in range(1, RMAX + 1):
                maybe_dma(r + 43, f1_dma, efw1_h, 'f')
                maybe_dma(r + 43, b3_dma, ebt3_h, 'b')
                if r == RMAX // 2:
                    nc.sync.dma_start(out=gcomb_sb[:], in_=gcomb_h[:])

                if r <= RF1:
                    wd = w[r] if r <= RF1 - 1 else P  # boundary MM full width
                    psf1 = ps_fw.tile([P, P], f32, tag="psf1")
                    nc.tensor.matmul(psf1[:, 0:wd], lhsT=m2_sb[:],
                                     rhs=v_st[:, 0:wd], start=True, stop=True)
                if r <= RB3:
                    s = S - r
                    wdy = w[s]
                    psb3 = ps_bw.tile([P, P], f32, tag="psb3")
                    nc.tensor.matmul(psb3[:, 0:wdy], lhsT=m2b_sb[:],
                                     rhs=y_st[:, 0:wdy], start=True, stop=True)
                if r <= RF2:
                    su = M1 + r - 1
                    wdu = w[su]
                    psf2 = ps_fw.tile([P, P], f32, tag="psf2")
                    nc.tensor.matmul(psf2[:, 0:wdu], lhsT=m2_sb[:],
                                     rhs=u_st[:, 0:wdu], start=True, stop=True)
                if r <= RB2:
                    sw = M2 - 1 - r
                    wdw = w[sw]
                    psb2 = ps_bw.tile([P, P], f32, tag="psb2")
                    nc.tensor.matmul(psb2[:, 0:wdw], lhsT=m2b_sb[:],
                                     rhs=w_st[:, 0:wdw], start=True, stop=True)

                if r <= RF1 - 1:
                    t, j = slot_of(f1_tiles, r)
                    nc.vector.tensor_tensor(out=v_st[:, 0:wd],
                                            in0=psf1[:, 0:wd],
                                            in1=t[:, j, 0:wd], op=ALU.mult)
                elif r == RF1:
                    ps_m1 = psf1
                if r <= RB3:
                    t, j = slot_of(b3_tiles, r)
                    nc.vector.tensor_tensor(out=y_st[:, 0:wdy],
                                            in0=psb3[:, 0:wdy],
                                            in1=t[:, j, 0:wdy], op=ALU.mult)
                if r <= RF2:
                    nc.vector.tensor_tensor(out=u_st[:, 0:wdu],
                                            in0=psf2[:, 0:wdu],
                                            in1=mid_sb[:, r - 1, 0:wdu],
                                            op=ALU.mult)
                    if r == RF2:
                        psu_m2 = ps_fw.tile([P, P], f32, tag="psf2")
                        nc.tensor.matmul(psu_m2[:], lhsT=m2_sb[:],
                                         rhs=u_st[:], start=True, stop=True)
                if r <= RB2:
                    nc.vector.tensor_tensor(out=w_st[:, 0:wdw],
                                            in0=psb2[:, 0:wdw],
                                            in1=mid_sb[:, NMID - 1 - r, 0:wdw],
                                            op=ALU.mult)

            # ---- epilogue ----
            nc.vector.tensor_reduce(gred[:], gcomb_sb[:], axis=AX.X, op=ALU.add)
            dA = dots.tile([P, P], f32)
            nc.vector.tensor_tensor(out=dA[:], in0=psu_m2[:], in1=y_st[:],
                                    op=ALU.mult)
            dC = dots.tile([P, P], f32)
            nc.vector.tensor_tensor(out=dC[:], in0=ps_m1[:], in1=w_st[:],
                                    op=ALU.mult)

            qA = ps_f.tile([G, P], f32, tag="psq")
            nc.tensor.matmul(qA[:], lhsT=gsel_sb[:], rhs=dA[:],
                             start=True, stop=True)
            lnA = small.tile([G, P], f32, tag="lnA")
            nc.scalar.activation(lnA[:], qA[:], AF.Ln)
            qC = ps_f.tile([G, P], f32, tag="psq")
            nc.tensor.matmul(qC[:], lhsT=gsel_sb[:], rhs=dC[:],
                             start=True, stop=True)
            lnC = small.tile([G, P], f32, tag="lnC")
            nc.scalar.activation(lnC[:], qC[:], AF.Ln)
            uf = dots.tile([P, P], f32)
            nc.scalar.copy(uf[:], u_st[:])
            qD = ps_f.tile([G, P], f32, tag="psq")
            nc.tensor.matmul(qD[:], lhsT=gsel_sb[:], rhs=uf[:],
                             start=True, stop=True)
            lnD = small.tile([G, P], f32, tag="lnD")
            nc.scalar.activation(lnD[:], qD[:], AF.Ln)

            fwd4 = small.tile([G, P], f32, tag="fwd4")
            nc.vector.tensor_add(fwd4[:], lnA[:], lnC[:])
            nc.vector.tensor_sub(fwd4[:], fwd4[:], lnD[:])
            fred = small.tile([G, 1], f32, tag="fred")
            nc.vector.tensor_reduce(fred[:], fwd4[:], axis=AX.X, op=ALU.add)

            psf1s = ps_f.tile([1, 1], f32, tag="pss")
            nc.tensor.matmul(psf1s[:], lhsT=fred[:], rhs=ones_sb[0:G, :],
                             start=True, stop=True)
            psg1 = ps_f.tile([1, 1], f32, tag="pss")
            nc.tensor.matmul(psg1[:], lhsT=gred[:], rhs=ones_sb[:],
                             start=True, stop=True)
            tf_sb = small.tile([1, 1], f32, tag="tf")
            nc.scalar.copy(tf_sb[:], psf1s[:])
            out_sb = small.tile([1, 1], f32, tag="outs")
            nc.vector.tensor_tensor(out=out_sb[:], in0=tf_sb[:], in1=psg1[:],
                                    op=ALU.subtract)
            nc.sync.dma_start(out=loss_h[:], in_=out_sb[:])

    return nc


def _estimate_k(feats, transitions):
    """Per-step log-growth of the forward recursion, from a 128-seq sample."""
    m = np.exp(transitions.T.astype(np.float64))  # m[frm, to]
    f = feats[:128].astype(np.float64)
    v = np.exp(transitions.T[START][None, :] + f[:, 0, :])
    v[:, 30:] = 0.0
    c = np.log(v.sum(1))
    v /= v.sum(1, keepdims=True)
    for s in range(1, S):
        v = (v @ m) * np.exp(f[:, s, :])
        v[:, 30:] = 0.0
        q = v.sum(1)
        c += np.log(q)
        v /= q[:, None]
    return float(c.mean() / S)


def _host_inputs(feats, tags, lengths, transitions, plan):
    import ml_dtypes
    bf16 = ml_dtypes.bfloat16
    f8 = ml_dtypes.float8_e5m2

    feats = np.asarray(feats, np.float32)
    tags = np.asarray(tags).astype(np.int64)
    lengths = np.asarray(lengths).astype(np.int64)
    transitions = np.asarray(transitions, np.float32)
    M1, M2 = plan["M1"], plan["M2"]

    K = _estimate_k(feats, transitions)

    # global longest-first deal: rank i -> core i%8, local r=i//8,
    # group r%4, column r//4  (columns die back-to-front, same on all cores)
    order = np.argsort(-lengths, kind="stable")
    perm = np.empty(B, np.int64)
    i = np.arange(B)
    core = i % NCORES
    lr = i // NCORES
    gg = lr % G
    col = lr // G
    perm[core * BC + gg * P + col] = order[i]
    feats = feats[perm]
    tags = tags[perm]
    lengths = lengths[perm]

    m = np.exp(transitions.T.astype(np.float64)).astype(np.float32)  # [frm,to]
    M2m = m.copy()
    M2m[:, STOP] = 1.0
    m2ext = np.zeros((P, P), np.float32)
    m2bext = np.zeros((P, P), np.float32)
    for g in range(G):
        m2ext[g * T:(g + 1) * T, g * T:(g + 1) * T] = M2m
        m2bext[g * T:(g + 1) * T, g * T:(g + 1) * T] = M2m.T
    m2ext = m2ext.astype(bf16)
    m2bext = m2bext.astype(bf16)

    gsel = np.zeros((P, G), np.float32)
    for g in range(G):
        gsel[g * T:(g + 1) * T, g] = 1.0

    rowt = np.arange(P) % T
    x2 = np.zeros((P, P), np.float32)
    x2[rowt <= 29, :] = 1.0
    x2 = x2.astype(bf16)

    flat = transitions.reshape(-1)
    tags_prev = np.concatenate(
        [np.full((B, 1), START, np.int64), tags[:, :-1]], axis=1)
    pairval = flat[(tags * T + tags_prev).reshape(-1)].reshape(B, S)
    emitval = np.take_along_axis(feats, tags[:, :, None], axis=2)[:, :, 0]
    smask = np.arange(S)[None, :] < lengths[:, None]
    gcomb = np.where(smask, pairval + emitval - K, 0.0).astype(np.float32)

    ef_all = np.exp(feats - np.float32(K))          # [B, S, T] f32
    init0 = np.exp(transitions.T[START][None, :] + feats[:, 0, :] - np.float32(K))
    init0[:, 30:] = 0.0

    per_core = []
    for c in range(NCORES):
        sl = slice(c * BC, (c + 1) * BC)
        len_c = lengths[sl]
        ef_c = ef_all[sl]
        eft = np.zeros((P, NSLOT, P), np.float32)
        src = ef_c.reshape(G, P, S, T).transpose(0, 3, 2, 1)
        eft_v = src.reshape(P, S, P)
        vmask = (np.arange(NSLOT)[None, :] < len_c[:, None])
        vm = vmask.reshape(G, P, NSLOT).transpose(0, 2, 1).reshape(
            G, 1, NSLOT, P) * np.ones((1, T, 1, 1))
        vm = vm.reshape(P, NSLOT, P)
        eft[:, 1:S, :] = np.where(vm[:, 1:S, :] > 0, eft_v[:, 1:S, :], 0.0)
        eft[rowt >= 30, :, :] = 0.0
        pad = (vm[:, :, :] == 0)
        r31 = (rowt == STOP)
        eft[np.ix_(r31, np.arange(NSLOT))] = np.where(
            pad[r31], 1.0, eft[r31])
        i0 = init0[sl].reshape(G, P, T).transpose(0, 2, 1).reshape(P, P)
        eft[:, 0, :] = i0
        eft8 = np.clip(eft, 0.0, 57344.0).astype(f8)
        per_core.append({
            "efw1": np.ascontiguousarray(eft8[:, 0:M1, :]),
            "mid": np.ascontiguousarray(eft8[:, M1:M2, :]),
            "ebt3": np.ascontiguousarray(eft8[:, S:M2 - 1:-1, :]),
            "x2": x2,
            "v0": np.ascontiguousarray(eft[:, 0, :].astype(bf16)),
            "y0": np.ascontiguousarray(eft[:, S, :].astype(bf16)),
            "w0": np.ascontiguousarray(eft[:, M2 - 1, :].astype(bf16)),
            "m2ext": m2ext,
            "m2bext": m2bext,
            "gsel": gsel,
            "gcomb": np.ascontiguousarray(
                gcomb[sl].reshape(G, P, S).transpose(1, 0, 2).reshape(P, G * S)),
        })
    return per_core


def kernel(feats, tags, lengths, transitions):
    global _compiled, _plan
    from concourse.bass_utils import run_bass_kernel_spmd
    import waitfix_embedded  # noqa: F401  (installs on import)

    if _plan is None:
        _plan = _make_plan(lengths)
    if _compiled is None:
        _compiled = _build_bass(_plan)
    nc = _compiled
    in_maps = _host_inputs(feats, tags, lengths, transitions, _plan)
    res = run_bass_kernel_spmd(nc, in_maps, core_ids=list(range(NCORES)))
    total = np.float64(0.0)
    for r in res.results:
        total += np.float64(r["loss_part"][0, 0])
    return np.float32(total / B)


# ---- embedded waitfix module (kernel.py must be self-contained) ----
import types as _types  # noqa: E402

_wf_src = '''
import json

MAX_WAITS = 1

def split_sync_waits(bir_bytes, max_waits=MAX_WAITS):
    bir = json.loads(bir_bytes)
    n_split = 0
    for fn in bir["functions"]:
        for blk in fn["blocks"]:
            out = []
            for inst in blk["instructions"]:
                si = inst.get("sync_info")
                waits = (si or {}).get("on_wait") or []
                if len(waits) > max_waits:
                    k = 0
                    while len(waits) > max_waits:
                        chunk, waits = waits[:max_waits], waits[max_waits:]
                        out.append({
                            "debug": inst.get("debug", 0),
                            "engine": inst["engine"],
                            "ins": [], "is_reset_sema": False,
                            "name": inst["name"] + "-wsplit%d" % k,
                            "opcode": "NoOp", "outs": [],
                            "sync_info": {"on_update": [], "on_wait": chunk},
                        })
                        k += 1
                    si["on_wait"] = waits
                    n_split += 1
                out.append(inst)
            blk["instructions"] = out
    return json.dumps(bir).encode()

def install():
    import concourse.bass2jax as bass2jax
    if getattr(bass2jax, "_waitfix_installed", False):
        return
    orig = bass2jax.compile_bir_kernel
    def patched(bir_json, tmpdir, neff_name="file.neff"):
        return orig(split_sync_waits(bir_json), tmpdir, neff_name)
    bass2jax.compile_bir_kernel = patched
    bass2jax._waitfix_installed = True

install()
'''
if "waitfix_embedded" not in sys.modules:
    _mod = _types.ModuleType("waitfix_embedded")
    exec(_wf_src, _mod.__dict__)
    sys.modules["waitfix_embedded"] = _mod


if __name__ == "__main__":
    import refcache
    inputs, exp = refcache.load()
    out = kernel(**inputs)
    rel = abs(float(out) - float(exp)) / max(abs(float(exp)), 1e-9)
    print("kernel:", out, "expected:", exp, "rel err:", rel)
